# revision 1
# baseline (speedup 1.0000x reference)
"""GCN layer (gather-scale-scatter + dense transform) on 8 trn2 NeuronCores.

out[r] = (sum_{e:row[e]=r} norm_e * x[col_e]  (incl self loop norm=dis^2)) @ W + bias
with norm_e = dis[row]*dis[col], dis = rsqrt(1 + outdeg).

Sharding: destination nodes partitioned across 8 cores (12500 each); each
core is fully independent (x is replicated input; no collectives).

Device algorithm per core:
  - edges grouped host-side by (dest block of 96, src block of 32768),
    padded to chunks of 128 edges, chunk counts equalized across cores so a
    single NEFF serves all 8 cores.
  - dma_gather pulls x[col] rows (512B) from HBM into SBUF slabs.
  - per chunk, DVE builds S[e,d] = (iota[d]==rowloc[e]) * norm[e] in one
    tensor_scalar op; PE accumulates aggT[f,d] += xg.T @ S in PSUM.
  - per dest block: aggT -> SBUF, out = aggT.T @ W + bias -> HBM.
  Dest-block width 96 balances PE time (4 cyc/row fp32 matmul, prop.
  to width) against gather padding (prop. to group count).
"""

import numpy as np

import concourse.bass as bass
import concourse.mybir as mybir
import concourse.tile as tile
from concourse import bacc
from concourse.bass_utils import run_bass_kernel_spmd

F = 128          # feature dim (in == out)
CH = 128         # edges per chunk
N_CORES = 8


def _prep(x, edge_index, n_nodes, src_blk, dblk=128):
    """Host-side integer/index preprocessing. Returns per-core device arrays
    and the static chunk-structure shared by all cores."""
    npc = n_nodes // N_CORES            # nodes per core
    nblk = (npc + dblk - 1) // dblk     # dest blocks per core
    nk = (n_nodes + src_blk - 1) // src_blk  # src blocks

    r = np.asarray(edge_index[0], dtype=np.int64)
    c = np.asarray(edge_index[1], dtype=np.int64)
    deg = (np.bincount(r, minlength=n_nodes) + 1).astype(np.float64)
    dis = (1.0 / np.sqrt(deg)).astype(np.float32)

    loops = np.arange(n_nodes, dtype=np.int64)
    rr = np.concatenate([r, loops])
    cc = np.concatenate([c, loops])
    norm = dis[rr] * dis[cc]

    core = rr // npc
    rloc = rr - core * npc
    b_arr = rloc // dblk                # dest block
    d_arr = (rloc % dblk).astype(np.float32)  # local dest within block
    k_arr = cc // src_blk

    # group counts per (core, b, k) -> equalized chunk counts Cbk [nblk, nk]
    gid = (core * nblk + b_arr) * nk + k_arr
    counts = np.bincount(gid, minlength=N_CORES * nblk * nk).reshape(
        N_CORES, nblk, nk
    )
    Cbk = np.ceil(counts / CH).astype(np.int64).max(axis=0)  # [nblk, nk]
    maxcnt = counts.max(axis=0)         # [nblk, nk] max real edges per group
    Ck_tot = Cbk.sum(axis=0)            # [nk] chunks per src block
    C_tot = int(Cbk.sum())

    # chunk base offsets
    chunk_base = np.zeros((nblk, nk), np.int64)   # in global (b-major) order
    chunk_base.reshape(-1)[1:] = np.cumsum(Cbk.reshape(-1))[:-1]
    kpos_base = np.zeros((nblk, nk), np.int64)    # per-k (b-major within k)
    kpos_base[1:, :] = np.cumsum(Cbk, axis=0)[:-1, :]

    per_core = []
    for ci in range(N_CORES):
        sel = core == ci
        bs, ks, ds, cs, ns = b_arr[sel], k_arr[sel], d_arr[sel], cc[sel], norm[sel]
        order = np.lexsort((cs, ks, bs))
        bs, ks, ds, cs, ns = bs[order], ks[order], ds[order], cs[order], ns[order]
        # position within (b,k) group
        g = bs * nk + ks
        cnt = np.bincount(g, minlength=nblk * nk)
        gstart = np.zeros(nblk * nk, np.int64)
        gstart[1:] = np.cumsum(cnt)[:-1]
        j = np.arange(len(g)) - gstart[g]

        rowloc = np.full(C_tot * CH, -1.0, np.float32)
        normv = np.zeros(C_tot * CH, np.float32)
        slot = chunk_base[bs, ks] * CH + j
        rowloc[slot] = ds
        normv[slot] = ns

        idx_ks = []
        for k in range(nk):
            arr = np.zeros(int(Ck_tot[k]) * CH, np.int16)
            # -1 tail beyond the cross-core max count: dma_gather skips the
            # transfer for trailing negative idxs (slot keeps stale data,
            # cancelled by rowloc=-1 in the segment matrix)
            maxc_rep = np.repeat(maxcnt[:, k], Cbk[:, k] * CH)
            base_rep = np.repeat(kpos_base[:, k] * CH, Cbk[:, k] * CH)
            posrel = np.arange(int(Ck_tot[k]) * CH) - base_rep
            arr[posrel >= maxc_rep] = -1
            m = ks == k
            kslot = kpos_base[bs[m], k] * CH + j[m]
            arr[kslot] = (cs[m] - k * src_blk).astype(np.int16)
            # wrap: idx i -> [i % 16, i // 16], replicated to 128 partitions
            wrapped = arr.reshape(-1, 16).T          # [16, Ck*8]
            idx_ks.append(np.tile(wrapped, (8, 1)).copy())  # [128, Ck*8]

        per_core.append(
            {
                "rowloc": rowloc.reshape(C_tot, CH).T.copy(),  # [128, C_tot]
                "normv": normv.reshape(C_tot, CH).T.copy(),
                **{f"idx{k}": idx_ks[k] for k in range(nk)},
            }
        )

    return per_core, Cbk, Ck_tot, C_tot, npc, nblk, nk, maxcnt


def _build(n_nodes, src_blk, Cbk, Ck_tot, C_tot, npc, nblk, nk, G, wide=False,
           dblk=128, maxcnt=None):
    """Build the Bass program (shared by all cores).

    wide=True: accumulate two dest blocks per PSUM tile ([128, 256]) and run
    the segment matmuls in float32r, which streams at 1 cyc/row when the
    moving dim is >=256 (vs 4 for fp32) -- 2x PE time per chunk saved.
    """
    nc = bacc.Bacc(None, target_bir_lowering=False)
    dt = mybir.dt
    assert dblk == 128 or not wide
    DW = 256 if wide else dblk  # psum/S width in dest columns

    mm_t = dt.float32r if wide else dt.float32
    x_d = nc.dram_tensor("x", [n_nodes, F], mm_t, kind="ExternalInput")
    w_d = nc.dram_tensor("w", [F, F], dt.float32, kind="ExternalInput")
    iota_d = nc.dram_tensor("iota", [128, DW], dt.float32, kind="ExternalInput")
    bias_d = nc.dram_tensor("biasrep", [128, F], dt.float32, kind="ExternalInput")
    rowloc_d = nc.dram_tensor("rowloc", [128, C_tot], dt.float32, kind="ExternalInput")
    normv_d = nc.dram_tensor("normv", [128, C_tot], dt.float32, kind="ExternalInput")
    idx_d = [
        nc.dram_tensor(f"idx{k}", [128, int(Ck_tot[k]) * 8], dt.int16,
                       kind="ExternalInput")
        for k in range(nk)
    ]
    y_d = nc.dram_tensor("y", [npc, F], dt.float32, kind="ExternalOutput")

    with tile.TileContext(nc) as tc:
        with (
            tc.tile_pool(name="const", bufs=1) as constp,
            tc.tile_pool(name="slab", bufs=10) as slabp,
            tc.tile_pool(name="idxp", bufs=10) as idxp,
            tc.tile_pool(name="sp", bufs=8) as sp,
            tc.tile_pool(name="pre", bufs=3) as prep,
            tc.tile_pool(name="ob", bufs=3) as obp,
            tc.tile_pool(name="pagg", bufs=4, space="PSUM") as paggp,
            tc.tile_pool(name="pout", bufs=2, space="PSUM") as poutp,
        ):
            w_sb = constp.tile([F, F], dt.float32, tag="w")
            nc.sync.dma_start(w_sb[:], w_d[:])
            iota_sb = constp.tile([128, DW], dt.float32, tag="iota")
            nc.sync.dma_start(iota_sb[:], iota_d[:])
            bias_sb = constp.tile([128, F], dt.float32, tag="bias")
            nc.sync.dma_start(bias_sb[:], bias_d[:])
            rowloc_sb = constp.tile([128, C_tot], dt.float32, tag="rowloc")
            nc.sync.dma_start(rowloc_sb[:], rowloc_d[:])
            normv_sb = constp.tile([128, C_tot], dt.float32, tag="normv")
            nc.sync.dma_start(normv_sb[:], normv_d[:])

            x_src = [x_d[k * src_blk: min((k + 1) * src_blk, n_nodes), :]
                     for k in range(nk)]

            # Pre-zero all slab slots: trailing -1 gather idxs skip the
            # transfer, so skipped slots read stale slot data; generation-0
            # slots would otherwise be uninitialized (NaN*0=NaN in PSUM).
            maxcbk = int(Cbk.max())
            for _ in range(10):
                t0 = slabp.tile([128, maxcbk, F], mm_t, tag="slab")
                z_ap = t0[:] if mm_t == dt.float32 else t0[:].bitcast(dt.float32)
                nc.vector.memset(z_ap, 0.0)

            BPG = 2 if wide else 1   # dest blocks per psum group
            S_BUFS = 8
            if wide:
                # Pre-zero every slot of each half-tag S pool. Chunks of an
                # even/odd block only ever write their own half of an
                # "s0"/"s1" tile, so the other half stays zero across slot
                # generations and the [128, DW] matmul read is safe.
                for h in range(BPG):
                    for _ in range(S_BUFS):
                        t0 = sp.tile([128, DW], mm_t, tag=f"s{h}")
                        nc.vector.memset(t0[:].bitcast(dt.float32), 0.0)

            pos = [0] * nk   # per-k consumed chunk count
            ci = 0           # global chunk index
            for g in range((nblk + BPG - 1) // BPG):
                blocks = [b for b in range(g * BPG, min((g + 1) * BPG, nblk))]
                pa = paggp.tile([128, DW], dt.float32, tag="pagg")
                nchunks = int(sum(Cbk[b].sum() for b in blocks))
                done = 0
                for b in blocks:
                    h = b - g * BPG
                    hs = h * (DW // BPG)
                    for k in range(nk):
                        cbk = int(Cbk[b, k])
                        if cbk == 0:
                            continue
                        # one gather per (b,k) group: trailing -1 idxs (the
                        # chunk-rounding pad beyond the max-core edge count)
                        # transfer nothing; num_idxs_reg = that max count,
                        # identical on every core by construction.
                        slab = slabp.tile([128, cbk, F], mm_t, tag="slab")
                        it = idxp.tile([128, cbk * 8], dt.int16, tag="idx")
                        base = pos[k]
                        nc.sync.dma_start(
                            it[:], idx_d[k][:, base * 8: (base + cbk) * 8]
                        )
                        nc.gpsimd.dma_gather(
                            slab[:],
                            x_src[k],
                            it[:],
                            cbk * CH,
                            int(maxcnt[b, k]),
                            F,
                            # >64 descs in one packet wedges the SDMA engine
                            single_packet=False,
                        )
                        pos[k] += cbk
                        for off in range(cbk):
                            s_t = sp.tile([128, DW], mm_t, tag=f"s{h}")
                            nc.vector.tensor_scalar(
                                s_t[:, hs: hs + dblk],
                                iota_sb[:, :dblk],
                                rowloc_sb[:, ci: ci + 1],
                                normv_sb[:, ci: ci + 1],
                                mybir.AluOpType.is_equal,
                                mybir.AluOpType.mult,
                            )
                            nc.tensor.matmul(
                                pa[:],
                                slab[:, off, :],
                                s_t[:],
                                start=(done == 0),
                                stop=(done == nchunks - 1),
                            )
                            ci += 1
                            done += 1
                # aggT [f, d-group] -> SBUF on scalar engine
                pre = prep.tile([128, DW], dt.float32, tag="pre")
                nc.scalar.activation(
                    pre[:], pa[:], mybir.ActivationFunctionType.Copy
                )
                for b in blocks:
                    h = b - g * BPG
                    hs = h * (DW // BPG)
                    bw = DW // BPG
                    po = poutp.tile([128, F], dt.float32, tag="pout")
                    nc.tensor.matmul(
                        po[:bw, :], pre[:, hs: hs + bw], w_sb[:],
                        start=True, stop=True
                    )
                    ob = obp.tile([128, F], dt.float32, tag="ob")
                    nc.vector.tensor_add(ob[:bw, :], po[:bw, :], bias_sb[:bw, :])
                    rows = min(bw, npc - b * bw)
                    nc.sync.dma_start(
                        y_d[b * bw: b * bw + rows, :], ob[:rows, :]
                    )

    nc.compile()
    return nc


def kernel(x, edge_index, weight, bias, _n_nodes=100000, _src_blk=32768, _g=16,
           _wide=False, _dblk=96, _return_nc=False):
    x = np.ascontiguousarray(np.asarray(x, dtype=np.float32))
    edge_index = np.asarray(edge_index)
    weight = np.ascontiguousarray(np.asarray(weight, dtype=np.float32))
    bias = np.asarray(bias, dtype=np.float32)
    n_nodes = x.shape[0]
    assert n_nodes == _n_nodes and n_nodes % N_CORES == 0

    per_core, Cbk, Ck_tot, C_tot, npc, nblk, nk, maxcnt = _prep(
        x, edge_index, n_nodes, _src_blk, dblk=_dblk
    )
    nc = _build(n_nodes, _src_blk, Cbk, Ck_tot, C_tot, npc, nblk, nk, _g,
                wide=_wide, dblk=_dblk, maxcnt=maxcnt)

    iota = np.tile(np.arange(256 if _wide else _dblk, dtype=np.float32), (128, 1))
    biasrep = np.tile(bias[None, :], (128, 1)).astype(np.float32)
    in_maps = [
        {
            "x": x,
            "w": weight,
            "iota": iota,
            "biasrep": biasrep,
            **per_core[ci],
        }
        for ci in range(N_CORES)
    ]
    res = run_bass_kernel_spmd(nc, in_maps, core_ids=list(range(N_CORES)))
    out = np.concatenate([res.results[ci]["y"] for ci in range(N_CORES)], axis=0)
    if _return_nc:
        return out, nc, in_maps
    return out



# revision 10
# speedup vs baseline: 10.4550x; 10.4550x over previous
"""GCN layer on 8 trn2 NeuronCores -- dual-path gather (DMA + GPSIMD).

out[r] = (sum_{e:row[e]=r} dis[row_e]*dis[col_e] * x[col_e]) @ W + bias,
dis = rsqrt(1 + outdeg), self-loops included as ordinary edges.

Sharding: destination nodes partitioned across 8 cores (12500 each); each
core independent (x replicated input; no collectives).

Device algorithm per core (norm fully factored out of the edge stream):
  - X' = x * dis[col] in bf16 (host). dis[row] applied at the output stage.
  - Edges routed to 6 flat streams: 4 DMA streams (col-block of 32768, int16
    gather idx) + 2 GPSIMD streams (col < 65536, served from an SBUF-resident
    pair-packed X' copy via ap_gather, then PE-transposed to edge-major).
  - Streams are grids per (superblock of 512 dests, stream), equalized
    across cores so one NEFF serves all 8; slots hold gathered bf16 rows.
  - Per 128-slot chunk and touched dest block: S = (iota == rowloc) 0/1
    indicator bf16, built 8 chunks per DVE tensor_tensor via broadcast APs;
    PE accumulates aggT[f,d] += slab.T @ S in PSUM per 128-dest block.
  - Drain per block: aggT->bf16 (Act), po = aggT.T @ W (PE),
    ob = po*dis_d + bias (DVE scalar_tensor_tensor), DMA to y.
"""

import numpy as np
import ml_dtypes

import concourse.bass as bass
import concourse.mybir as mybir
import concourse.tile as tile
from concourse import bacc
from concourse import library_config
from concourse.bass_utils import run_bass_kernel_spmd

F = 128
CH = 128          # edges per chunk (slab partition dim)
N_CORES = 8
NPC = 12500       # dest nodes per core
DBLK = 128        # dest block width (psum tile)
NBLK = (NPC + DBLK - 1) // DBLK   # 98
SUP = 512         # superblock of dests (grid granularity)
NSUP = (NPC + SUP - 1) // SUP     # 25
KBLK = 32768      # dma-path col blocking (int16 idx limit)
NKD = 4           # dma streams
RES_ELEMS = 32768  # ap_gather num_elems per half (f32 pair elements)
RES_NODES = 2 * RES_ELEMS  # 65536 cols covered by the resident
SB = 8            # chunks per S8 tensor_tensor batch
GB = 4096         # edges per dma_gather batch
PB = 1024         # edges per ap_gather batch (8 transpose windows)


def _prep(x, edge_index, weight, bias, pool_frac):
    """Host-side routing/index prep. Returns (shared structure, per-core arrays)."""
    n = x.shape[0]
    r = np.asarray(edge_index[0], dtype=np.int64)
    c = np.asarray(edge_index[1], dtype=np.int64)
    deg = (np.bincount(r, minlength=n) + 1).astype(np.float64)
    dis = (1.0 / np.sqrt(deg)).astype(np.float32)

    loops = np.arange(n, dtype=np.int64)
    rr = np.concatenate([r, loops])
    cc = np.concatenate([c, loops])

    xs = (x * dis[:, None]).astype(ml_dtypes.bfloat16)   # X' [n, 128] bf16
    # pair-packed resident: [128, RES_ELEMS] f32; partition p (=64*h+q) holds
    # f32 word q of X'[u] for nodes u in half h's range split across q... see
    # below: half h serves cols [h*32768, (h+1)*32768), channels h*64..h*64+63,
    # element u = node h*32768+u, partition q = feature pair q.
    resv = np.zeros((128, RES_ELEMS), dtype=np.float32)
    for h in range(2):
        blk = xs[h * KBLK:(h + 1) * KBLK].view(np.float32)  # [32768, 64]
        resv[h * 64:(h + 1) * 64, :] = blk.T

    core = rr // NPC
    rloc = rr - core * NPC

    NS = NKD + 2
    # per (core, sup, stream) edge lists
    order = np.lexsort((cc, rloc, core))
    rr_s, cc_s, rl_s, co_s = rr[order], cc[order], rloc[order], core[order]
    sup_s = rl_s // SUP

    rng = np.random.RandomState(12345)
    kblk = cc_s // KBLK
    # sup 0 stays DMA-only so the pool path never stalls on the resident load
    elig = (cc_s < RES_NODES) & (sup_s >= 1)
    topool = elig & (rng.rand(len(cc_s)) < pool_frac)
    stream = np.where(topool, NKD + kblk, np.minimum(kblk, NKD - 1))
    # (col >= 98304 goes to k=3 stream; idx offset handled per stream)

    # counts [core, sup, stream]
    idx3 = (co_s * NSUP + sup_s) * NS + stream
    cnt = np.bincount(idx3, minlength=N_CORES * NSUP * NS).reshape(
        N_CORES, NSUP, NS)
    grid = cnt.max(axis=0)          # [NSUP, NS] shared slot allocation
    base = np.zeros((NSUP, NS), np.int64)
    base[1:, :] = np.cumsum(grid, axis=0)[:-1, :]
    L = grid.sum(axis=0)            # stream lengths
    # pad stream lengths: dma to %128, pool to %PB
    Lp = np.empty(NS, np.int64)
    for s in range(NS):
        m = CH if s < NKD else PB
        Lp[s] = max(((L[s] + m - 1) // m) * m, m)

    nchunks = [int(Lp[s] // CH) for s in range(NS)]

    # per-core slot assignment (position within (core, sup, stream) group,
    # preserving the rloc-sorted order within each group)
    gstart = np.zeros(N_CORES * NSUP * NS, np.int64)
    gcnt = np.bincount(idx3, minlength=N_CORES * NSUP * NS)
    gstart[1:] = np.cumsum(gcnt)[:-1]
    order2 = np.argsort(idx3, kind="stable")
    within = np.empty(len(rr_s), np.int64)
    within[order2] = np.arange(len(rr_s)) - gstart[idx3[order2]]
    slot = base[sup_s, stream] + within   # slot within stream

    # events: (stream, chunk, block) union over cores
    # per-core b-ranges per (stream, chunk): compute via edge slots
    bset = set()
    b_s = rl_s // DBLK
    chunk_of = slot // CH
    for key in zip(stream, chunk_of, b_s):
        bset.add(key)
    events = sorted(bset, key=lambda t: (t[2], t[0], t[1]))  # by (b, s, ci)
    NEV = len(events)
    NEVp = ((NEV + SB - 1) // SB) * SB

    # first/last event index per block
    ev_of_b = {}
    for j, (s, ci, b) in enumerate(events):
        ev_of_b.setdefault(b, []).append(j)

    # per-core device arrays
    per_core = []
    ev_index = {(s, ci, b): j for j, (s, ci, b) in enumerate(events)}
    for ci_ in range(N_CORES):
        sel = co_s == ci_
        st_c, sl_c, cc_c, rl_c, b_c = (stream[sel], slot[sel], cc_s[sel],
                                       rl_s[sel], b_s[sel])
        idx_arrs = []
        for s in range(NS):
            arr = np.zeros(int(Lp[s]), np.int16)
            m = st_c == s
            off = KBLK * (s - NKD) if s >= NKD else KBLK * min(s, NKD - 1)
            v = cc_c[m] - off
            if s == NKD - 1:  # k=3 stream also holds col >= 98304
                v = np.minimum(v, KBLK - 1)  # safety; cols < 100000-98304+32768 ok
            arr[sl_c[m]] = v.astype(np.int16)
            # wrap 16 + replicate
            wrapped = arr.reshape(-1, 16).T.copy()  # [16, Lp/16]
            rep = 8 if s < NKD else 4
            idx_arrs.append(np.tile(wrapped, (rep, 1)).copy())

        rowloc = np.full((NEVp, CH), -1.0, dtype=np.float32)
        jj = np.array([ev_index[(s_, sl_ // CH, b_)]
                       for s_, sl_, b_ in zip(st_c, sl_c, b_c)])
        rowloc[jj, sl_c % CH] = (rl_c - b_c * DBLK).astype(np.float32)
        disdst = np.zeros((DBLK, NBLK), np.float32)
        dcore = dis[ci_ * NPC:(ci_ + 1) * NPC]
        for b in range(NBLK):
            seg = dcore[b * DBLK: b * DBLK + DBLK]
            disdst[:len(seg), b] = seg
        per_core.append({
            "rowloc": np.ascontiguousarray(
                rowloc.T.astype(ml_dtypes.bfloat16)),  # [128, NEVp] bf16
            "disdst": disdst,                          # [128, NBLK] f32
            **{f"idx{s}": idx_arrs[s] for s in range(NS)},
        })

    iotab = np.tile(np.arange(DBLK, dtype=np.float32), (128, SB)
                    ).astype(ml_dtypes.bfloat16)       # [128, SB*128]
    shared = {
        "xs": np.ascontiguousarray(xs),
        "resv": resv,
        "wb": weight.astype(ml_dtypes.bfloat16),
        "iotab": iotab,
        "biasrep": np.tile(bias.astype(np.float32)[None, :], (DBLK, 1)),
        "eye64": np.eye(64, dtype=np.float32),
    }
    struct = {
        "events": events, "NEV": NEV, "NEVp": NEVp, "Lp": Lp,
        "nchunks": nchunks, "NS": NS,
    }
    return struct, shared, per_core


def _build(struct):
    events, NEVp, Lp, nchunks, NS = (struct["events"], struct["NEVp"],
                                     struct["Lp"], struct["nchunks"],
                                     struct["NS"])
    nc = bacc.Bacc(None, target_bir_lowering=False)
    dt = mybir.dt

    xs_d = nc.dram_tensor("xs", [100000, F], dt.bfloat16, kind="ExternalInput")
    resv_d = nc.dram_tensor("resv", [128, RES_ELEMS], dt.float32,
                            kind="ExternalInput")
    w_d = nc.dram_tensor("wb", [F, F], dt.bfloat16, kind="ExternalInput")
    iota_d = nc.dram_tensor("iotab", [128, SB * DBLK], dt.bfloat16,
                            kind="ExternalInput")
    bias_d = nc.dram_tensor("biasrep", [DBLK, F], dt.float32,
                            kind="ExternalInput")
    rowloc_d = nc.dram_tensor("rowloc", [128, NEVp], dt.bfloat16,
                              kind="ExternalInput")
    disdst_d = nc.dram_tensor("disdst", [DBLK, NBLK], dt.float32,
                              kind="ExternalInput")
    eye_d = nc.dram_tensor("eye64", [64, 64], dt.float32, kind="ExternalInput")
    idx_d = [nc.dram_tensor(f"idx{s}", [128 if s < NKD else 64,
                                        int(Lp[s]) // 16], dt.int16,
                            kind="ExternalInput")
             for s in range(NS)]
    y_d = nc.dram_tensor("y", [NPC, F], dt.float32, kind="ExternalOutput")

    CPB_D = GB // CH   # dma chunks per slab batch
    CPB_P = PB // CH   # pool chunks per batch
    have_pool = any(s >= NKD for s, _, _ in events)

    with tile.TileContext(nc) as tc:
        with (
            tc.tile_pool(name="const", bufs=1) as constp,
            tc.tile_pool(name="slab", bufs=3) as slabp,
            tc.tile_pool(name="idxp", bufs=3) as idxp,
            tc.tile_pool(name="pidxp", bufs=3) as pidxp,
            tc.tile_pool(name="slabt", bufs=3) as slabtp,
            tc.tile_pool(name="slab8", bufs=3) as slab8p,
            tc.tile_pool(name="sp", bufs=3) as sp_,
            tc.tile_pool(name="pre", bufs=3) as prep,
            tc.tile_pool(name="ob", bufs=3) as obp,
            tc.tile_pool(name="ptr", bufs=2, space="PSUM") as ptrp,
            tc.tile_pool(name="pagg", bufs=4, space="PSUM") as paggp,
            tc.tile_pool(name="pout", bufs=2, space="PSUM") as poutp,
        ):
            if have_pool:
                nc.gpsimd.load_library(library_config.ap_gather)
            w_sb = constp.tile([F, F], dt.bfloat16, tag="w")
            nc.sync.dma_start(w_sb[:], w_d[:])
            iota_sb = constp.tile([128, SB * DBLK], dt.bfloat16, tag="iota")
            nc.sync.dma_start(iota_sb[:], iota_d[:])
            bias_sb = constp.tile([DBLK, F], dt.float32, tag="bias")
            nc.sync.dma_start(bias_sb[:], bias_d[:])
            rowloc_sb = constp.tile([128, NEVp], dt.bfloat16, tag="rowloc")
            nc.sync.dma_start(rowloc_sb[:], rowloc_d[:])
            disdst_sb = constp.tile([DBLK, NBLK], dt.float32, tag="disdst")
            nc.sync.dma_start(disdst_sb[:], disdst_d[:])
            if have_pool:
                res_sb = constp.tile([128, RES_ELEMS], dt.float32, tag="res")
                nc.sync.dma_start(res_sb[:], resv_d[:])
                eye_sb = constp.tile([64, 64], dt.float32, tag="eye")
                nc.sync.dma_start(eye_sb[:], eye_d[:])

            x_src = [xs_d[min(s, NKD - 1) * KBLK:
                          min((min(s, NKD - 1) + 1) * KBLK, 100000), :]
                     for s in range(NKD)]

            # stream state
            nbatch_done = [0] * NS
            slab_tiles = {}   # (s, batch) -> (tile, kind)

            def ensure_batch(s, bi):
                if nbatch_done[s] > bi:
                    return
                assert nbatch_done[s] == bi, (s, bi, nbatch_done[s])
                nbatch_done[s] += 1
                if s < NKD:
                    n_idx = min(GB, int(Lp[s]) - bi * GB)
                    n_ch = (n_idx + CH - 1) // CH
                    slab = slabp.tile([128, CPB_D, F], dt.bfloat16, tag="slab")
                    it = idxp.tile([128, GB // 16], dt.int16, tag="idx")
                    nc.sync.dma_start(
                        it[:, :n_idx // 16],
                        idx_d[s][:, bi * (GB // 16):
                                 bi * (GB // 16) + n_idx // 16])
                    nc.gpsimd.dma_gather(
                        slab[:, :n_ch, :], x_src[s], it[:, :n_idx // 16],
                        n_idx, n_idx, F, single_packet=False)
                    slab_tiles[(s, bi)] = slab
                else:
                    h = s - NKD
                    pt = pidxp.tile([64, PB // 16], dt.int16, tag="pidx")
                    nc.sync.dma_start(
                        pt[:], idx_d[s][:, bi * (PB // 16):
                                        (bi + 1) * (PB // 16)])
                    st = slabtp.tile([128, PB], dt.float32, tag="slabt")
                    sl = st[h * 64:(h + 1) * 64, :]
                    nc.gpsimd.ap_gather(
                        sl, res_sb[h * 64:(h + 1) * 64, :], pt[:],
                        64, RES_ELEMS, 1, PB)
                    ptr = ptrp.tile([128, CPB_P, 64], dt.float32, tag="ptr")
                    for w in range(CPB_P):
                        nc.tensor.transpose(
                            ptr[:, w, :], sl[:, w * CH:(w + 1) * CH],
                            eye_sb[:])
                    s8 = slab8p.tile([128, CPB_P * 64], dt.float32, tag="s8")
                    nc.scalar.activation(
                        s8[:], ptr[:].rearrange("p a b -> p (a b)"),
                        mybir.ActivationFunctionType.Copy)
                    slab_tiles[(s, bi)] = s8

            def slab_slice(s, ci):
                cpb = CPB_D if s < NKD else CPB_P
                bi = ci // cpb
                t = slab_tiles[(s, bi)]
                if s < NKD:
                    return t[:, ci % cpb, :]
                w = ci % cpb
                return t[:, w * 64:(w + 1) * 64].bitcast(dt.bfloat16)

            # block -> event index span (events sorted by (b, s, ci))
            ev_b = [e[2] for e in events]
            s8_tiles = {}

            def ensure_s8(g):
                if g in s8_tiles:
                    return
                t = sp_.tile([128, SB * DBLK], dt.bfloat16, tag="s8t")
                nc.vector.tensor_tensor(
                    t[:].rearrange("p (a b) -> p a b", a=SB),
                    iota_sb[:].rearrange("p (a b) -> p a b", a=SB),
                    rowloc_sb[:, g * SB:(g + 1) * SB]
                    .unsqueeze(2).broadcast_to([128, SB, DBLK]),
                    mybir.AluOpType.is_equal)
                s8_tiles[g] = t

            j = 0
            import os
            NEV = struct["NEV"]
            evlimit = int(os.environ.get("EVLIMIT", "0"))
            if evlimit:
                NEV = min(NEV, evlimit)
            while j < NEV:
                b = ev_b[j]
                j_end = j
                while j_end < NEV and ev_b[j_end] == b:
                    j_end += 1
                pa = paggp.tile([128, DBLK], dt.float32, tag="pagg")
                for jj in range(j, j_end):
                    s, ci, _ = events[jj]
                    cpb = CPB_D if s < NKD else CPB_P
                    ensure_batch(s, ci // cpb)
                    g = jj // SB
                    ensure_s8(g)
                    st8 = s8_tiles[g]
                    nc.tensor.matmul(
                        pa[:], slab_slice(s, ci),
                        st8[:, (jj % SB) * DBLK:(jj % SB + 1) * DBLK],
                        start=(jj == j), stop=(jj == j_end - 1))
                # drain block b
                pre = prep.tile([128, DBLK], dt.bfloat16, tag="pre")
                nc.scalar.activation(pre[:], pa[:],
                                     mybir.ActivationFunctionType.Copy)
                po = poutp.tile([DBLK, F], dt.float32, tag="po")
                nc.tensor.matmul(po[:], pre[:], w_sb[:], start=True, stop=True)
                ob = obp.tile([DBLK, F], dt.float32, tag="ob")
                nc.vector.scalar_tensor_tensor(
                    ob[:], po[:], disdst_sb[:, b:b + 1], bias_sb[:],
                    op0=mybir.AluOpType.mult, op1=mybir.AluOpType.add)
                rows = min(DBLK, NPC - b * DBLK)
                nc.sync.dma_start(y_d[b * DBLK: b * DBLK + rows, :],
                                  ob[:rows, :])
                j = j_end

    nc.compile()
    return nc


def kernel(x, edge_index, weight, bias, _pool_frac=0.72, _return_nc=False):
    x = np.ascontiguousarray(np.asarray(x, dtype=np.float32))
    edge_index = np.asarray(edge_index)
    weight = np.ascontiguousarray(np.asarray(weight, dtype=np.float32))
    bias = np.asarray(bias, dtype=np.float32)
    n = x.shape[0]
    assert n == 100000 and n % N_CORES == 0

    struct, shared, per_core = _prep(x, edge_index, weight, bias, _pool_frac)
    nc = _build(struct)

    in_maps = [{**shared, **per_core[ci]} for ci in range(N_CORES)]
    res = run_bass_kernel_spmd(nc, in_maps, core_ids=list(range(N_CORES)))
    out = np.concatenate([res.results[ci]["y"] for ci in range(N_CORES)],
                         axis=0)
    if _return_nc:
        return out, nc, in_maps
    return out
